# revision 22
# baseline (speedup 1.0000x reference)
"""SO3Conv Trainium2 Bass kernel.

Math (per reference):
  psi[f,g,i] = sum_n D[n,i] w[f,g,n] / sqrt(64)
  per l (d=2l+1, blk=d*d at offset off):
    y[b,g,off+v*d+m] = 1/sqrt(64*d) * sum_{f,u} x[b,f,off+u*d+m] * psi[f,g,off+u*d+v]

Strategy: data-parallel over batch (8 cores x 128 batch).
Per core, all matmul operands in bf16 (fp32 PSUM accumulate):
  A) psi computed on PE in "psiT" layout [(u,v)-part, (f,g)-free], then
     reshaped via SBUF->SBUF DMA into per-(l,ku) rhs tiles
     [(u,f)-part, (v,g)-free]  (K-chunks ku = pairs of u, 2*64=128 rows).
  B) x loaded contiguously (SWDGE cast fp32->bf16), transposed on PE per
     (l, ku, m) into lhsT tiles [(u,f)-part, b-free].
  C) matmuls accumulate over ku into PSUM [b, (v,g)], copied (cast bf16,
     scattered) into full y in natural layout, stored with SWDGE cast
     bf16->fp32.
"""

import sys

sys.path.insert(0, "/opt/trn_rl_repo")

import numpy as np

LMAX = 6
F = 64
NROT = 64
IRREP = 455
B = 1024
NCORES = 8
BS = B // NCORES  # 128

DS = [2 * l + 1 for l in range(LMAX + 1)]
OFFS = []
_o = 0
for _d in DS:
    OFFS.append(_o)
    _o += _d * _d
assert _o == IRREP

_CACHE = {}


def _build():
    import concourse.bacc as bacc
    import concourse.bass as bass
    import concourse.mybir as mybir
    from concourse import tile

    dt = mybir.dt
    BF = dt.bfloat16
    F32 = dt.float32

    nc = bacc.Bacc("TRN2", target_bir_lowering=False, debug=False, num_devices=NCORES)

    x_d = nc.dram_tensor("x", [BS, F, IRREP], F32, kind="ExternalInput")
    D_d = nc.dram_tensor("D", [NROT, IRREP], F32, kind="ExternalInput")
    w_d = nc.dram_tensor("w", [F, F, NROT], F32, kind="ExternalInput")
    id_d = nc.dram_tensor("ident", [128, 128], BF, kind="ExternalInput")
    y_d = nc.dram_tensor("y", [BS, F, IRREP], F32, kind="ExternalOutput")
    # DRAM scratch for the psi layout shuffle: S[i, (f,g)]
    s_d = nc.dram_tensor("psiS", [IRREP, F * F], BF)

    with tile.TileContext(nc) as tc:
        with (
            tc.tile_pool(name="big", bufs=1) as big,
            tc.tile_pool(name="rhs", bufs=1) as rhsp,
            tc.tile_pool(name="const", bufs=1) as cp,
            tc.tile_pool(name="pt", bufs=2, space=bass.MemorySpace.PSUM) as pt,
            tc.tile_pool(name="py", bufs=4, space=bass.MemorySpace.PSUM) as py,
        ):
            # ---- persistent SBUF ----
            x_bf = big.tile([BS, F, IRREP], BF)
            y_bf = big.tile([BS, F, IRREP], BF)
            ident = cp.tile([128, 128], BF)
            nc.sync.dma_start(ident[:, :], id_d[:, :])

            # rhs tiles per (l, ku):  [krows, d*64] bf16, free idx = v*64+g
            rhs = {}
            for l in range(LMAX + 1):
                d = DS[l]
                nku = (d + 1) // 2
                for ku in range(nku):
                    nu = 2 if (ku * 2 + 1) < d else 1
                    rhs[(l, ku)] = rhsp.tile([nu * 64, d * 64], BF, name=f"rhs{l}_{ku}", tag=f"rhs{l}_{ku}")

            # ---- load D (scaled 1/8, bf16) and w (bf16) ----
            d_f32 = cp.tile([NROT, IRREP], F32)
            nc.sync.dma_start(d_f32[:, :], D_d[:, :])
            d_bf = cp.tile([NROT, IRREP], BF)
            nc.scalar.mul(d_bf[:, :], d_f32[:, :], 1.0 / 8.0)

            # w (f,g,n) -> w_bf [128, 32, 64] : partition p, chunk c of (f*64+g)=c*128+p
            w_bf = cp.tile([128, 32, NROT], BF)
            w_view = w_d.rearrange("f g n -> (f g) n").rearrange(
                "(c p) n -> p c n", p=128
            )
            nc.gpsimd.dma_start(w_bf[:, :, :], w_view)

            # ---- x load (SWDGE cast): l=6 now; the rest after psi phase ----
            _mid6 = (OFFS[6] + IRREP) // 2
            _mid5 = (OFFS[5] + OFFS[6]) // 2
            for i0, i1 in ((OFFS[6], _mid6), (_mid6, IRREP)):
                nc.gpsimd.dma_start(x_bf[:, :, i0:i1], x_d[:, :, i0:i1])

            # wT [n=64, (f,g)=4096] via PE transposes
            wT = cp.tile([NROT, F * F], BF)
            for cgrp in range(4):  # 8 transposes per psum bank
                ps = pt.tile([128, 1024], BF, tag="ptx", name="psw")
                for t in range(8):
                    c = cgrp * 8 + t
                    nc.tensor.transpose(
                        ps[:64, t * 128 : (t + 1) * 128], w_bf[:, c, :], ident[:, :]
                    )
                nc.vector.tensor_copy(
                    wT[:, cgrp * 1024 : (cgrp + 1) * 1024], ps[:64, :]
                )

            # ---- psi in psiT layout + reshape to rhs tiles ----
            # psiT chunk rows r = flat (u*d+v) index within l-block (<=128 rows)
            s_fvg = s_d.rearrange("i (f g) -> f i g", g=64)
            with (
                tc.tile_pool(name="lhs", bufs=1) as lp,
                tc.tile_pool(name="psit", bufs=2) as psp,
                tc.tile_pool(name="pa", bufs=2, space=bass.MemorySpace.PSUM) as pa,
            ):
                eng_flip = 0
                for l in range(LMAX, -1, -1):
                    d = DS[l]
                    blk = d * d
                    off = OFFS[l]
                    norm = 1.0 / np.sqrt(64.0 * d)
                    r0 = 0
                    while r0 < blk:
                        rows = min(128, blk - r0)
                        psiT = psp.tile([128, F * F], BF, tag="psiT")
                        for s in range(8):
                            pps = pa.tile([128, 512], F32, tag="ptp", name="pps")
                            nc.tensor.matmul(
                                pps[:rows, :],
                                d_bf[:, off + r0 : off + r0 + rows],
                                wT[:, s * 512 : (s + 1) * 512],
                                start=True,
                                stop=True,
                            )
                            dst = psiT[:rows, s * 512 : (s + 1) * 512]
                            if eng_flip % 2 == 0:
                                nc.scalar.mul(dst, pps[:rows, :], norm)
                            else:
                                nc.vector.tensor_scalar_mul(dst, pps[:rows, :], norm)
                            eng_flip += 1
                        # park this chunk in DRAM scratch (contiguous rows)
                        nc.sync.dma_start(
                            s_d[off + r0 : off + r0 + rows, :], psiT[:rows, :]
                        )
                        r0 += rows
                    # read back with (f, v, g)-ordered APs into rhs tiles
                    for u in range(d):
                        ku, uin = divmod(u, 2)
                        src_ap = s_fvg[:, off + u * d : off + (u + 1) * d, :]
                        dst = rhs[(l, ku)][uin * 64 : (uin + 1) * 64, :].rearrange(
                            "f (v g) -> f v g", g=64
                        )
                        nc.sync.dma_start(dst, src_ap)

                # ---- rest of x (after psi DMAs in priority order) ----
                for i0, i1 in (
                    (OFFS[5], _mid5),
                    (_mid5, OFFS[6]),
                    (OFFS[4], OFFS[5]),
                    (OFFS[3], OFFS[4]),
                    (0, OFFS[3]),
                ):
                    nc.gpsimd.dma_start(x_bf[:, :, i0:i1], x_d[:, :, i0:i1])

                # ---- main loop ----
                for l in range(LMAX, -1, -1):
                    d = DS[l]
                    off = OFFS[l]
                    nku = (d + 1) // 2
                    if d * 64 <= 512:
                        vsplits = [(0, d)]
                    else:
                        vh = (d + 1) // 2
                        vsplits = [(0, vh), (vh, d - vh)]

                    xv = x_bf[:, :, off : off + d * d].rearrange(
                        "b f (u m) -> b u f m", u=d
                    )
                    lts = []
                    for ku in range(nku):
                        nu = 2 if (ku * 2 + 1) < d else 1
                        lt = lp.tile(
                            [nu * 64, d * 128], BF, tag=f"lhsT{ku}", name=f"lt{l}_{ku}"
                        )
                        lts.append(lt)
                        for m0 in range(0, d, 8):
                            mm = min(8, d - m0)
                            ps = pt.tile([128, 1024], BF, tag="ptx", name="psx")
                            for t in range(mm):
                                m = m0 + t
                                for uin in range(nu):
                                    src = xv[:, 2 * ku + uin, :, m]
                                    nc.tensor.transpose(
                                        ps[
                                            uin * 64 : (uin + 1) * 64,
                                            t * 128 : (t + 1) * 128,
                                        ],
                                        src,
                                        ident[:, :],
                                    )
                            nc.vector.tensor_copy(
                                lt[:, m0 * 128 : (m0 + mm) * 128],
                                ps[: nu * 64, : mm * 128],
                            )
                    yv = y_bf[:, :, off : off + d * d].rearrange(
                        "b g (v m) -> b v g m", v=d
                    )
                    for m in range(d):
                        for v0, nv in vsplits:
                            pyt = py.tile([BS, 512], F32, tag="py", name="pyt")
                            out = pyt[:, : nv * 64]
                            for ku in range(nku):
                                nc.tensor.matmul(
                                    out,
                                    lts[ku][:, m * 128 : (m + 1) * 128],
                                    rhs[(l, ku)][:, v0 * 64 : (v0 + nv) * 64],
                                    start=(ku == 0),
                                    stop=(ku == nku - 1),
                                )
                            dst = yv[:, v0 : v0 + nv, :, m]
                            src = out.rearrange("b (v g) -> b v g", g=64)
                            if (m + v0) % 2 == 0:
                                nc.scalar.copy(dst, src)
                            else:
                                nc.vector.tensor_copy(dst, src)

            # ---- store y (SWDGE cast bf16->fp32), 4 col-chunks ----
            yflat_s = y_bf.rearrange("b f i -> b (f i)")
            yflat_d = y_d.rearrange("b f i -> b (f i)")
            CH = F * IRREP // 4
            for c in range(4):
                nc.gpsimd.dma_start(
                    yflat_d[:, c * CH : (c + 1) * CH], yflat_s[:, c * CH : (c + 1) * CH]
                )

    nc.compile()
    return nc


def _get_nc():
    if "nc" not in _CACHE:
        _CACHE["nc"] = _build()
    return _CACHE["nc"]


def kernel(x, D, w):
    import ml_dtypes
    from concourse.bass_utils import run_bass_kernel_spmd

    nc = _get_nc()
    ident = np.eye(128, dtype=ml_dtypes.bfloat16)
    in_maps = [
        {
            "x": np.ascontiguousarray(x[c * BS : (c + 1) * BS]),
            "D": np.ascontiguousarray(D),
            "w": np.ascontiguousarray(w),
            "ident": ident,
        }
        for c in range(NCORES)
    ]
    res = run_bass_kernel_spmd(nc, in_maps, core_ids=list(range(NCORES)))
    out = np.concatenate([r["y"] for r in res.results], axis=0)
    return out.astype(np.float32)
